# revision 5
# baseline (speedup 1.0000x reference)
"""Domain discrepancy (MMD-style) loss kernel for 8 Trainium2 NeuronCores.

reference computes, for S, T in R^{4096 x 2048}:
    k(x, y) = exp(-||x - y||^2 / d^2),   d = 2048
    out = mean(Kss) + mean(Ktt) - 2 * mean(Kst)        (float32 scalar)

Strategy
--------
All kernel arguments z = -||x-y||^2/d^2 lie within ~1.2e-3 of z0 = -2/d, so
k = exp(z0) * e^w with w = z - z0, |w| <~ 1e-3.  A 2nd-order Taylor expansion
of e^w is exact to ~1e-16 per element, which turns the three kernel-matrix
means into
    sum_ij k = c * (N*M + Sum(w) + Sum(w^2)/2),   c = exp(z0)
with w_ij = 2*<x_i, y_j>/d^2 + hb_i + hc_j, hb_i = (d - ||x_i||^2)/d^2.
Sum(w) and the bias cross-terms of Sum(w^2) collapse to O(N*D) analytic sums
(host, float64); only Sum_ij <x_i,y_j>^2 needs the pairwise matrices.

The device therefore runs the three 4096x4096x2048 GEMMs in fp8 (e4m3) with
DoubleRow perf mode and reduces each PSUM tile with one VectorE bn_stats op
(count/mean/M2 give Sum(ps) and Sum(ps^2) per row).  Row sharding: core c owns
512 rows of S and T as the stationary operand (each weight tile is reused by
8 consecutive matmuls over the moving j-tiles); the full S^T/T^T moving
operands stay resident in SBUF (fp8, 16 MB).

The final three means are combined in float32 exactly like the reference
(xx + yy - 2*xy on fp32-rounded means), reproducing its arithmetic.
"""

import numpy as np
import ml_dtypes
from contextlib import ExitStack

import concourse.bass as bass
import concourse.tile as tile
from concourse import bacc, mybir
from concourse import bass_utils

N, D = 4096, 2048
NCORES = 8
RPC = N // NCORES          # rows per core (stationary side)
IC = RPC // 128            # 4 stationary i-chunks of 128
JT = N // 512              # 8 moving j-tiles of 512
KB = D // 128              # 16 contraction chunks of 128
KK = KB // 2               # 8 DoubleRow steps of 256
SCALE = float(2.0 / (D * D))
F32 = mybir.dt.float32
FP8 = mybir.dt.float8e4

_compiled = {}


def _build():
    nc = bacc.Bacc("TRN2", target_bir_lowering=False, debug=False,
                   num_devices=NCORES)

    # moving operands: full matrices, k-chunk-major   mov[kk, p, i2*N + j]
    mov_s = nc.dram_tensor("mov_s", [KK, 128, 2 * N], FP8, kind="ExternalInput")
    mov_t = nc.dram_tensor("mov_t", [KK, 128, 2 * N], FP8, kind="ExternalInput")
    # stationary operands: this core's rows           rows[p, k*RPC + i]
    rows_s = nc.dram_tensor("rows_s", [128, KB * RPC], FP8, kind="ExternalInput")
    rows_t = nc.dram_tensor("rows_t", [128, KB * RPC], FP8, kind="ExternalInput")
    out = nc.dram_tensor("out", [128, 3 * IC * JT * 6], F32, kind="ExternalOutput")

    with tile.TileContext(nc) as tc, ExitStack() as ctx:
        const_pool = ctx.enter_context(tc.tile_pool(name="const", bufs=1))
        psum_pool = ctx.enter_context(tc.tile_pool(name="psum", bufs=8, space="PSUM"))

        ms = const_pool.tile([128, KB * N], FP8, tag="ms")
        mt = const_pool.tile([128, KB * N], FP8, tag="mt")
        # chunked loads so compute can start after the first k-chunks land
        for kk in range(KK):
            nc.sync.dma_start(ms[:, kk * 2 * N:(kk + 1) * 2 * N], mov_s.ap()[kk])
        for kk in range(KK):
            nc.sync.dma_start(mt[:, kk * 2 * N:(kk + 1) * 2 * N], mov_t.ap()[kk])
        rs = const_pool.tile([128, KB * RPC], FP8, tag="rs")
        nc.sync.dma_start(rs[:], rows_s.ap())
        rt = const_pool.tile([128, KB * RPC], FP8, tag="rt")
        nc.sync.dma_start(rt[:], rows_t.ap())
        out_sb = const_pool.tile([128, 3 * IC * JT * 6], F32, tag="out_sb")

        for mat, (rows, mov) in enumerate([(rs, ms), (rt, mt), (rs, mt)]):
            rows3 = rows[:].rearrange("p (k i) -> p k i", k=KB)
            mov3 = mov[:].rearrange("p (k j) -> p k j", k=KB)
            for ic in range(IC):
                pss = [psum_pool.tile([128, 512], F32, tag="ps", name=f"ps_{mat}_{ic}_{jt}")
                       for jt in range(JT)]
                for kk in range(KK):
                    w_ap = rows3[:, 2 * kk:2 * kk + 2, ic * 128:(ic + 1) * 128]
                    for jt in range(JT):
                        nc.tensor.matmul(
                            pss[jt][:], w_ap,
                            mov3[:, 2 * kk:2 * kk + 2, jt * 512:(jt + 1) * 512],
                            start=(kk == 0), stop=(kk == KK - 1),
                            perf_mode=mybir.MatmulPerfMode.DoubleRow,
                        )
                for jt in range(JT):
                    col = ((mat * IC + ic) * JT + jt) * 6
                    nc.vector.bn_stats(out_sb[:, col:col + 6], pss[jt][:])
        nc.sync.dma_start(out.ap(), out_sb[:])

    nc.compile()
    return nc


def _get_nc():
    if "nc" not in _compiled:
        _compiled["nc"] = _build()
    return _compiled["nc"]


def _prep_inputs(S, T):
    """Host-side shard/layout prep (float32 -> fp8 e4m3, transposed tilings)."""
    Sb = S.astype(ml_dtypes.float8_e4m3)
    Tb = T.astype(ml_dtypes.float8_e4m3)

    def mov(X):
        # mov[kk, p, i2*N + j] = X[j, 128*(2kk+i2)+p]
        return np.ascontiguousarray(
            X.reshape(N, KK, 2, 128).transpose(1, 3, 2, 0)
        ).reshape(KK, 128, 2 * N)

    def rows(X, c):
        # r[p, k*RPC+i] = X[c*RPC+i, 128k+p]
        blk = X[c * RPC:(c + 1) * RPC]
        return np.ascontiguousarray(
            blk.reshape(RPC, KB, 128).transpose(2, 1, 0)
        ).reshape(128, KB * RPC)

    movS, movT = mov(Sb), mov(Tb)
    in_maps = []
    for c in range(NCORES):
        in_maps.append({
            "mov_s": movS, "mov_t": movT,
            "rows_s": rows(Sb, c), "rows_t": rows(Tb, c),
        })
    return in_maps, Sb, Tb


def _combine(per_core_outs, S, T, Sb, Tb):
    """Host float64 combination of device partial sums -> the three means."""
    S64, T64 = S.astype(np.float64), T.astype(np.float64)
    Sq64, Tq64 = Sb.astype(np.float64), Tb.astype(np.float64)
    x2 = (S64 ** 2).sum(1)
    y2 = (T64 ** 2).sum(1)
    hbS = (D - x2) / (D * D)
    hbT = (D - y2) / (D * D)
    sSq = Sq64.sum(0)
    sTq = Tq64.sum(0)

    # decode bn_stats -> Sum_ij ps^2 per matrix (summed over cores/tiles/rows)
    Bsum = np.zeros(3)
    for o in per_core_outs:
        o = o.astype(np.float64).reshape(128, 3 * IC * JT, 6)
        m_e, v_e = o[:, :, 1], o[:, :, 2]
        m_o, v_o = o[:, :, 4], o[:, :, 5]
        sq = v_e + 256.0 * m_e ** 2 + v_o + 256.0 * m_o ** 2   # [128, tiles]
        sq = sq.reshape(128, 3, IC * JT).sum(axis=2)           # per matrix
        Bsum += sq.sum(axis=0)

    cfg = [
        (hbS, hbS, Sq64, Sq64, sSq, sSq),   # xx
        (hbT, hbT, Tq64, Tq64, sTq, sTq),   # yy
        (hbS, hbT, Sq64, Tq64, sSq, sTq),   # xy: i-side S, j-side T
    ]
    c0 = np.exp(-2.0 / D)
    s = SCALE
    means = []
    for mat, (hb, hc, U, V, sU, sV) in enumerate(cfg):
        Sw = s * (sU @ sV) + N * hb.sum() + N * hc.sum()
        Sw2 = (s * s * Bsum[mat] + N * (hb ** 2).sum() + N * (hc ** 2).sum()
               + 2.0 * hb.sum() * hc.sum()
               + 2.0 * s * (hb @ (U @ sV) + hc @ (V @ sU)))
        means.append(c0 * (1.0 + (Sw + 0.5 * Sw2) / (float(N) * N)))
    return means


def kernel(source_features, target_features):
    S = np.asarray(source_features, dtype=np.float32)
    T = np.asarray(target_features, dtype=np.float32)

    nc = _get_nc()
    in_maps, Sb, Tb = _prep_inputs(S, T)
    import os
    trace = bool(int(os.environ.get("BASS_KERNEL_TRACE", "0")))
    res = bass_utils.run_bass_kernel_spmd(
        nc, in_maps, core_ids=list(range(NCORES)), trace=trace)
    _compiled["last_results"] = res
    per_core = [np.asarray(r["out"], np.float32) for r in res.results]

    means = _combine(per_core, S, T, Sb, Tb)
    f = np.float32
    xx, yy, xy = (f(m) for m in means)
    val = f(f(xx + yy) - f(2.0) * xy)
    return np.array(val, dtype=np.float32)


# revision 6
# speedup vs baseline: 1.2087x; 1.2087x over previous
"""Domain discrepancy (MMD-style) loss kernel for 8 Trainium2 NeuronCores.

reference computes, for S, T in R^{4096 x 2048}:
    k(x, y) = exp(-||x - y||^2 / d^2),   d = 2048
    out = mean(Kss) + mean(Ktt) - 2 * mean(Kst)        (float32 scalar)

Strategy
--------
All kernel arguments z = -||x-y||^2/d^2 lie within ~1.2e-3 of z0 = -2/d, so
k = exp(z0) * e^w with w = z - z0, |w| <~ 1e-3.  A 2nd-order Taylor expansion
of e^w is exact to ~1e-16 per element, which turns the three kernel-matrix
means into
    sum_ij k = c * (N*M + Sum(w) + Sum(w^2)/2),   c = exp(z0)
with w_ij = 2*<x_i, y_j>/d^2 + hb_i + hc_j, hb_i = (d - ||x_i||^2)/d^2.
Sum(w) and the bias cross-terms of Sum(w^2) collapse to O(N*D) analytic sums
(host, float64); only Sum_ij <x_i,y_j>^2 needs the pairwise matrices.

The device therefore runs the three 4096x4096x2048 GEMMs in fp8 (e4m3) with
DoubleRow perf mode and reduces each PSUM tile with one VectorE bn_stats op
(count/mean/M2 give Sum(ps) and Sum(ps^2) per row).  Row sharding per the
spec hint: core c owns 512 rows of S and T (the moving operand); the full
S^T/T^T stream through as stationary j-chunks.

The final three means are combined in float32 exactly like the reference
(xx + yy - 2*xy on fp32-rounded means), reproducing its arithmetic.
"""

import numpy as np
import ml_dtypes
from contextlib import ExitStack

import concourse.bass as bass
import concourse.tile as tile
from concourse import bacc, mybir
from concourse import bass_utils

N, D = 4096, 2048
NCORES = 8
RPC = N // NCORES          # rows per core (moving-operand width)
MB = N // 128              # 32 stationary j-chunks of 128
KB = D // 128              # 16 contraction chunks of 128
KK = KB // 2               # 8 DoubleRow steps of 256
SCALE = float(2.0 / (D * D))
F32 = mybir.dt.float32
FP8 = mybir.dt.float8e4

_compiled = {}


def _build():
    nc = bacc.Bacc("TRN2", target_bir_lowering=False, debug=False,
                   num_devices=NCORES)

    lhs_s = nc.dram_tensor("lhs_s", [MB, 128, KB * 128], FP8, kind="ExternalInput")
    lhs_t = nc.dram_tensor("lhs_t", [MB, 128, KB * 128], FP8, kind="ExternalInput")
    rhs_s = nc.dram_tensor("rhs_s", [128, KB * RPC], FP8, kind="ExternalInput")
    rhs_t = nc.dram_tensor("rhs_t", [128, KB * RPC], FP8, kind="ExternalInput")
    out = nc.dram_tensor("out", [128, 3 * MB * 6], F32, kind="ExternalOutput")

    with tile.TileContext(nc) as tc, ExitStack() as ctx:
        const_pool = ctx.enter_context(tc.tile_pool(name="const", bufs=1))
        slab_pool = ctx.enter_context(tc.tile_pool(name="slabs", bufs=4))
        psum_pool = ctx.enter_context(tc.tile_pool(name="psum", bufs=8, space="PSUM"))

        rs = const_pool.tile([128, KB * RPC], FP8, tag="rs")
        nc.sync.dma_start(rs[:], rhs_s.ap())
        rt = const_pool.tile([128, KB * RPC], FP8, tag="rt")
        nc.sync.dma_start(rt[:], rhs_t.ap())
        out_sb = const_pool.tile([128, 3 * MB * 6], F32, tag="out_sb")

        lhs_s_ap = lhs_s.ap()
        lhs_t_ap = lhs_t.ap()
        for m in range(MB):
            slab_s = slab_pool.tile([128, KB * 128], FP8, tag="slab_s")
            nc.sync.dma_start(slab_s[:], lhs_s_ap[m])
            slab_t = slab_pool.tile([128, KB * 128], FP8, tag="slab_t")
            nc.sync.dma_start(slab_t[:], lhs_t_ap[m])
            # mat 0: xx (j over S, i over core's S rows)
            # mat 1: yy (j over T, i over core's T rows)
            # mat 2: xy (j over T, i over core's S rows)
            for mat, (slab, rhs) in enumerate(
                    [(slab_s, rs), (slab_t, rt), (slab_t, rs)]):
                ps = psum_pool.tile([128, RPC], F32, tag="ps",
                                    name=f"ps_{mat}_{m}")
                slab3 = slab[:].rearrange("p (k q) -> p k q", k=KB)
                rhs3 = rhs[:].rearrange("p (k i) -> p k i", k=KB)
                for kk in range(KK):
                    nc.tensor.matmul(
                        ps[:],
                        slab3[:, 2 * kk:2 * kk + 2, :],
                        rhs3[:, 2 * kk:2 * kk + 2, :],
                        start=(kk == 0), stop=(kk == KK - 1),
                        perf_mode=mybir.MatmulPerfMode.DoubleRow,
                    )
                col = (mat * MB + m) * 6
                nc.vector.bn_stats(out_sb[:, col:col + 6], ps[:])
        nc.sync.dma_start(out.ap(), out_sb[:])

    nc.compile()
    return nc


def _get_nc():
    if "nc" not in _compiled:
        _compiled["nc"] = _build()
    return _compiled["nc"]


def _prep_inputs(S, T):
    """Host-side shard/layout prep (float32 -> fp8 e4m3, transposed tilings)."""
    Sb = S.astype(ml_dtypes.float8_e4m3)
    Tb = T.astype(ml_dtypes.float8_e4m3)

    def slabs(X):
        # slab[m, p, k*128+q] = X[128m+q, 128k+p]
        return np.ascontiguousarray(
            X.reshape(MB, 128, KB, 128).transpose(0, 3, 2, 1)
        ).reshape(MB, 128, KB * 128)

    def rows(X, c):
        # r[p, k*RPC+i] = X[c*RPC+i, 128k+p]
        blk = X[c * RPC:(c + 1) * RPC]
        return np.ascontiguousarray(
            blk.reshape(RPC, KB, 128).transpose(2, 1, 0)
        ).reshape(128, KB * RPC)

    lhsS, lhsT_ = slabs(Sb), slabs(Tb)
    in_maps = []
    for c in range(NCORES):
        in_maps.append({
            "lhs_s": lhsS, "lhs_t": lhsT_,
            "rhs_s": rows(Sb, c), "rhs_t": rows(Tb, c),
        })
    return in_maps, Sb, Tb


def _combine(per_core_outs, S, T, Sb, Tb):
    """Host float64 combination of device partial sums -> the three means."""
    S64, T64 = S.astype(np.float64), T.astype(np.float64)
    Sq64, Tq64 = Sb.astype(np.float64), Tb.astype(np.float64)
    x2 = (S64 ** 2).sum(1)
    y2 = (T64 ** 2).sum(1)
    hbS = (D - x2) / (D * D)
    hbT = (D - y2) / (D * D)
    sSq = Sq64.sum(0)
    sTq = Tq64.sum(0)

    # decode bn_stats -> Sum_ij ps^2 per matrix (summed over cores/tiles/rows)
    Bsum = np.zeros(3)
    for o in per_core_outs:
        o = o.astype(np.float64).reshape(128, 3 * MB, 6)
        m_e, v_e = o[:, :, 1], o[:, :, 2]
        m_o, v_o = o[:, :, 4], o[:, :, 5]
        sq = v_e + 256.0 * m_e ** 2 + v_o + 256.0 * m_o ** 2   # [128, tiles]
        Bsum += sq.reshape(128, 3, MB).sum(axis=2).sum(axis=0)

    cfg = [
        (hbS, hbS, Sq64, Sq64, sSq, sSq),   # xx
        (hbT, hbT, Tq64, Tq64, sTq, sTq),   # yy
        (hbS, hbT, Sq64, Tq64, sSq, sTq),   # xy: i-side S, j-side T
    ]
    c0 = np.exp(-2.0 / D)
    s = SCALE
    means = []
    for mat, (hb, hc, U, V, sU, sV) in enumerate(cfg):
        Sw = s * (sU @ sV) + N * hb.sum() + N * hc.sum()
        Sw2 = (s * s * Bsum[mat] + N * (hb ** 2).sum() + N * (hc ** 2).sum()
               + 2.0 * hb.sum() * hc.sum()
               + 2.0 * s * (hb @ (U @ sV) + hc @ (V @ sU)))
        means.append(c0 * (1.0 + (Sw + 0.5 * Sw2) / (float(N) * N)))
    return means


def kernel(source_features, target_features):
    S = np.asarray(source_features, dtype=np.float32)
    T = np.asarray(target_features, dtype=np.float32)

    nc = _get_nc()
    in_maps, Sb, Tb = _prep_inputs(S, T)
    import os
    trace = bool(int(os.environ.get("BASS_KERNEL_TRACE", "0")))
    res = bass_utils.run_bass_kernel_spmd(
        nc, in_maps, core_ids=list(range(NCORES)), trace=trace)
    _compiled["last_results"] = res
    per_core = [np.asarray(r["out"], np.float32) for r in res.results]

    means = _combine(per_core, S, T, Sb, Tb)
    f = np.float32
    xx, yy, xy = (f(m) for m in means)
    val = f(f(xx + yy) - f(2.0) * xy)
    return np.array(val, dtype=np.float32)


# revision 7
# speedup vs baseline: 1.6398x; 1.3567x over previous
"""Domain discrepancy (MMD-style) loss kernel for 8 Trainium2 NeuronCores.

reference computes, for S, T in R^{4096 x 2048}:
    k(x, y) = exp(-||x - y||^2 / d^2),   d = 2048
    out = mean(Kss) + mean(Ktt) - 2 * mean(Kst)        (float32 scalar)

Strategy
--------
All kernel arguments z = -||x-y||^2/d^2 lie within ~1.2e-3 of z0 = -2/d, so
k = exp(z0) * e^w with w = z - z0, |w| <~ 1e-3.  A 2nd-order Taylor expansion
of e^w is exact to ~1e-16 per element, which turns the three kernel-matrix
means into
    sum_ij k = c * (N*M + Sum(w) + Sum(w^2)/2),   c = exp(z0)
with w_ij = 2*<x_i, y_j>/d^2 + hb_i + hc_j, hb_i = (d - ||x_i||^2)/d^2.
Sum(w) and the bias cross-terms of Sum(w^2) collapse to O(N*D) analytic sums
(host, float64); only Sum_ij <x_i,y_j>^2 needs the pairwise matrices.

All three Gram-squared sums live inside the symmetric 8192x8192 pairwise
matrix of Z = [S; T]: only its upper-triangle 512x512 blocks are computed —
136 block-GEMMs instead of the 192 a direct 3-matrix pass needs (-29% PE
work).  Each core gets 17 blocks (row-pair P=c with P=15-c balances the
triangle exactly).  GEMMs run in fp8 (e4m3) DoubleRow; each PSUM tile is
reduced by one VectorE bn_stats op (count/mean/M2 -> Sum(ps), Sum(ps^2)).
The host routes each block's sum to xx/yy/xy (P,Q<8 -> xx, P,Q>=8 -> yy,
mixed -> xy, off-diagonal blocks doubled) and assembles the three means in
float64.

The final means are combined in float32 exactly like the reference
(xx + yy - 2*xy on fp32-rounded means), reproducing its arithmetic.
"""

import numpy as np
import ml_dtypes
from contextlib import ExitStack

import concourse.bass as bass
import concourse.tile as tile
from concourse import bacc, mybir
from concourse import bass_utils

N, D = 4096, 2048
NCORES = 8
NB = 16                    # 512-row blocks of Z (8192 rows)
TPC = 17                   # triangle blocks per core
IC = 4                     # 128-row i-chunks per block
KB = D // 128              # 16 contraction chunks of 128
KK = KB // 2               # 8 DoubleRow steps of 256
SCALE = float(2.0 / (D * D))
F32 = mybir.dt.float32
FP8 = mybir.dt.float8e4

_compiled = {}


def blocks_for_core(c):
    out = [(c, q) for q in range(c, NB)]
    out += [(NB - 1 - c, q) for q in range(NB - 1 - c, NB)]
    return out


def _build():
    nc = bacc.Bacc("TRN2", target_bir_lowering=False, debug=False,
                   num_devices=NCORES)

    sta_all = nc.dram_tensor("sta_all", [TPC, 128, KB * 512], FP8, kind="ExternalInput")
    mov_all = nc.dram_tensor("mov_all", [TPC, 128, KB * 512], FP8, kind="ExternalInput")
    out = nc.dram_tensor("out", [128, TPC * IC * 6], F32, kind="ExternalOutput")

    with tile.TileContext(nc) as tc, ExitStack() as ctx:
        const_pool = ctx.enter_context(tc.tile_pool(name="const", bufs=1))
        slab_pool = ctx.enter_context(tc.tile_pool(name="slabs", bufs=4))
        psum_pool = ctx.enter_context(tc.tile_pool(name="psum", bufs=8, space="PSUM"))

        out_sb = const_pool.tile([128, TPC * IC * 6], F32, tag="out_sb")
        sta_ap = sta_all.ap()
        mov_ap = mov_all.ap()

        for t in range(TPC):
            sta = slab_pool.tile([128, KB * 512], FP8, tag="sta")
            nc.sync.dma_start(sta[:], sta_ap[t])
            mov = slab_pool.tile([128, KB * 512], FP8, tag="mov")
            nc.sync.dma_start(mov[:], mov_ap[t])
            sta3 = sta[:].rearrange("p (k i) -> p k i", k=KB)
            mov3 = mov[:].rearrange("p (k j) -> p k j", k=KB)
            for ic in range(IC):
                ps = psum_pool.tile([128, 512], F32, tag="ps", name=f"ps_{t}_{ic}")
                for kk in range(KK):
                    nc.tensor.matmul(
                        ps[:],
                        sta3[:, 2 * kk:2 * kk + 2, ic * 128:(ic + 1) * 128],
                        mov3[:, 2 * kk:2 * kk + 2, :],
                        start=(kk == 0), stop=(kk == KK - 1),
                        perf_mode=mybir.MatmulPerfMode.DoubleRow,
                    )
                col = (t * IC + ic) * 6
                nc.vector.bn_stats(out_sb[:, col:col + 6], ps[:])
        nc.sync.dma_start(out.ap(), out_sb[:])

    nc.compile()
    return nc


def _get_nc():
    if "nc" not in _compiled:
        _compiled["nc"] = _build()
    return _compiled["nc"]


def _prep_inputs(S, T):
    """Host-side shard/layout prep (float32 -> fp8 e4m3, transposed tilings)."""
    Sb = S.astype(ml_dtypes.float8_e4m3)
    Tb = T.astype(ml_dtypes.float8_e4m3)
    Zq = np.vstack([Sb, Tb])

    def rows(P):
        # r[p, k*512+i] = Z[P*512+i, 128k+p]
        blk = Zq[P * 512:(P + 1) * 512]
        return np.ascontiguousarray(
            blk.reshape(512, KB, 128).transpose(2, 1, 0)
        ).reshape(128, KB * 512)

    tiles = [rows(P) for P in range(NB)]
    in_maps = []
    for c in range(NCORES):
        blks = blocks_for_core(c)
        in_maps.append({
            "sta_all": np.stack([tiles[P] for P, _ in blks]),
            "mov_all": np.stack([tiles[Q] for _, Q in blks]),
        })
    return in_maps, Sb, Tb


def _combine(per_core_outs, S, T, Sb, Tb):
    """Host float64 combination of device partial sums -> the three means."""
    S64, T64 = S.astype(np.float64), T.astype(np.float64)
    Sq64, Tq64 = Sb.astype(np.float64), Tb.astype(np.float64)
    x2 = (S64 ** 2).sum(1)
    y2 = (T64 ** 2).sum(1)
    hbS = (D - x2) / (D * D)
    hbT = (D - y2) / (D * D)
    sSq = Sq64.sum(0)
    sTq = Tq64.sum(0)

    # decode bn_stats -> per-block Sum(ps^2), route to xx/yy/xy
    Bsum = np.zeros(3)
    for c, o in enumerate(per_core_outs):
        o = o.astype(np.float64).reshape(128, TPC * IC, 6)
        sq = (o[:, :, 2] + 256.0 * o[:, :, 1] ** 2
              + o[:, :, 5] + 256.0 * o[:, :, 4] ** 2)
        sq = sq.sum(axis=0).reshape(TPC, IC).sum(axis=1)
        for t, (P, Q) in enumerate(blocks_for_core(c)):
            if P < 8 and Q < 8:
                Bsum[0] += sq[t] * (1.0 if P == Q else 2.0)
            elif P >= 8 and Q >= 8:
                Bsum[1] += sq[t] * (1.0 if P == Q else 2.0)
            else:
                Bsum[2] += sq[t]

    cfg = [
        (hbS, hbS, Sq64, Sq64, sSq, sSq),   # xx
        (hbT, hbT, Tq64, Tq64, sTq, sTq),   # yy
        (hbS, hbT, Sq64, Tq64, sSq, sTq),   # xy: i-side S, j-side T
    ]
    c0 = np.exp(-2.0 / D)
    s = SCALE
    means = []
    for mat, (hb, hc, U, V, sU, sV) in enumerate(cfg):
        Sw = s * (sU @ sV) + N * hb.sum() + N * hc.sum()
        Sw2 = (s * s * Bsum[mat] + N * (hb ** 2).sum() + N * (hc ** 2).sum()
               + 2.0 * hb.sum() * hc.sum()
               + 2.0 * s * (hb @ (U @ sV) + hc @ (V @ sU)))
        means.append(c0 * (1.0 + (Sw + 0.5 * Sw2) / (float(N) * N)))
    return means


def kernel(source_features, target_features):
    S = np.asarray(source_features, dtype=np.float32)
    T = np.asarray(target_features, dtype=np.float32)

    nc = _get_nc()
    in_maps, Sb, Tb = _prep_inputs(S, T)
    import os
    trace = bool(int(os.environ.get("BASS_KERNEL_TRACE", "0")))
    res = bass_utils.run_bass_kernel_spmd(
        nc, in_maps, core_ids=list(range(NCORES)), trace=trace)
    _compiled["last_results"] = res
    per_core = [np.asarray(r["out"], np.float32) for r in res.results]

    means = _combine(per_core, S, T, Sb, Tb)
    f = np.float32
    xx, yy, xy = (f(m) for m in means)
    val = f(f(xx + yy) - f(2.0) * xy)
    return np.array(val, dtype=np.float32)
